# revision 17
# baseline (speedup 1.0000x reference)
"""CrossViewContrastiveLoss Trainium2 kernel.

loss = f(v1^T @ v2) where v1, v2 are [131072, 256] fp32 and f is a cheap
normalize/log epilogue on the [256, 256] joint matrix.

Strategy (data-parallel over N across 8 cores):
  - core c computes partial_c = v1[c*16384:(c+1)*16384]^T @ v2[same rows]
    as a PE GEMM streaming 32 MiB of HBM per core (memory-bound).
  - host sums the eight 256x256 partials in float64 and runs the epilogue
    (65536 elements -- negligible next to 256 MiB of streaming).
"""

import os

import numpy as np

import concourse.bacc as bacc
import concourse.bass as bass
import concourse.mybir as mybir
import concourse.tile as tile
from concourse import bass_utils

N_FULL = 131072
K = 256
NCORES = 8
N_LOC = N_FULL // NCORES  # 16384 rows per core
P = 128
NT = N_LOC // P  # 128 k-tiles of 128 rows per core
CHUNK = int(os.environ.get("CVCL_CHUNK", "8"))  # k-tiles per DMA
ALPHA = 9.0
EPS = 2.220446049250313e-16

# matmul input dtype mode: "bf16", "f32r" (fast fp32 modes), or "f32"
MM_MODE = os.environ.get("CVCL_MM_MODE", "bf16")
# input DMA queue assignment: "split" (v1 sync / v2 gpsimd) or "gpsimd"
DMA_Q = os.environ.get("CVCL_DMA_Q", "split")

_BUILD_CACHE = {}
LAST_RESULT = None  # BassKernelResults of the most recent run (for test.py)


def _install_axon_hooks_shim():
    """bass_utils' trace path imports antenv.axon_hooks, which this image
    lacks. Provide it, wiring the ctypes NTFF hook from trn_boot when the
    axon .so supports it. Harmless no-op when tracing is off."""
    import sys
    import types

    try:
        from antenv import axon_hooks  # noqa: F401

        return
    except ImportError:
        pass
    try:
        import antenv
    except ImportError:
        return
    mod = types.ModuleType("antenv.axon_hooks")
    mod._hook = None
    mod._resolved = False

    def set_axon_ntff_profile_hook(h):
        mod._hook = h
        mod._resolved = True

    def get_axon_ntff_profile_hook():
        # lazy: only touch the axon .so when tracing is actually requested
        if not mod._resolved:
            mod._resolved = True
            try:
                from trn_agent_boot.trn_boot import _ntff_profile_via_ctypes

                so_path = "/opt/axon/libaxon_pjrt.so"
                if os.path.exists(so_path):
                    mod._hook = _ntff_profile_via_ctypes(so_path)
            except Exception:
                mod._hook = None
        return mod._hook

    mod.set_axon_ntff_profile_hook = set_axon_ntff_profile_hook
    mod.get_axon_ntff_profile_hook = get_axon_ntff_profile_hook
    sys.modules["antenv.axon_hooks"] = mod
    antenv.axon_hooks = mod


try:
    _install_axon_hooks_shim()
except Exception:
    pass


def _build(mode):
    key = (mode, DMA_Q, CHUNK)
    if key in _BUILD_CACHE:
        return _BUILD_CACHE[key]

    nc = bacc.Bacc(
        "TRN2", target_bir_lowering=False, debug=False, num_devices=NCORES
    )
    v1 = nc.dram_tensor("v1", [N_LOC, K], mybir.dt.float32, kind="ExternalInput")
    v2 = nc.dram_tensor("v2", [N_LOC, K], mybir.dt.float32, kind="ExternalInput")
    out = nc.dram_tensor("partial", [K, K], mybir.dt.float32, kind="ExternalOutput")

    # [n, k] -> [p, t, k]: k-tile t holds rows t*128 .. t*128+127 on partitions
    v1r = v1.ap().rearrange("(t p) k -> p t k", p=P)
    v2r = v2.ap().rearrange("(t p) k -> p t k", p=P)
    out_ap = out.ap()

    mm_dt = {
        "f32r": mybir.dt.float32r,
        "bf16": mybir.dt.bfloat16,
        "f32": mybir.dt.float32,
    }[mode]

    with tile.TileContext(nc) as tc:
        with (
            tc.tile_pool(name="io", bufs=3) as io_pool,
            tc.tile_pool(name="cv", bufs=3) as cv_pool,
            tc.tile_pool(name="acc", bufs=1, space="PSUM") as psum_pool,
            tc.tile_pool(name="res", bufs=1) as res_pool,
        ):
            # one PSUM bank per 128-row chunk of the [256, 256] output
            ps0 = psum_pool.tile([P, K], mybir.dt.float32)
            ps1 = psum_pool.tile([P, K], mybir.dt.float32)

            nchunks = NT // CHUNK
            for ci in range(nchunks):
                sl = slice(ci * CHUNK, (ci + 1) * CHUNK)
                if mode in ("f32r", "bf16"):
                    # matmul inputs must be rounded by a compute op: DMA raw
                    # fp32 (two queues), then cast v1 on ACT / v2 on DVE.
                    raw1 = io_pool.tile([P, CHUNK, K], mybir.dt.float32, tag="r1")
                    raw2 = io_pool.tile([P, CHUNK, K], mybir.dt.float32, tag="r2")
                    q1 = nc.sync if DMA_Q == "split" else nc.gpsimd
                    q1.dma_start(raw1[:], v1r[:, sl, :])
                    nc.gpsimd.dma_start(raw2[:], v2r[:, sl, :])
                    t1 = cv_pool.tile([P, CHUNK, K], mm_dt, tag="c1")
                    t2 = cv_pool.tile([P, CHUNK, K], mm_dt, tag="c2")
                    nc.scalar.copy(t1[:], raw1[:])
                    nc.vector.tensor_copy(t2[:], raw2[:])
                else:
                    t1 = io_pool.tile([P, CHUNK, K], mybir.dt.float32, tag="r1")
                    t2 = io_pool.tile([P, CHUNK, K], mybir.dt.float32, tag="r2")
                    nc.sync.dma_start(t1[:], v1r[:, sl, :])
                    nc.gpsimd.dma_start(t2[:], v2r[:, sl, :])
                for j in range(CHUNK):
                    first = ci == 0 and j == 0
                    last = ci == nchunks - 1 and j == CHUNK - 1
                    rhs = t2[:, j, :]
                    nc.tensor.matmul(
                        ps0[:],
                        t1[:, j, 0:128],
                        rhs,
                        start=first,
                        stop=last,
                    )
                    nc.tensor.matmul(
                        ps1[:],
                        t1[:, j, 128:256],
                        rhs,
                        start=first,
                        stop=last,
                    )

            res = res_pool.tile([P, 2, K], mybir.dt.float32)
            nc.vector.tensor_copy(res[:, 0, :], ps0[:])
            nc.vector.tensor_copy(res[:, 1, :], ps1[:])
            nc.sync.dma_start(out_ap[0:128, :], res[:, 0, :])
            nc.sync.dma_start(out_ap[128:256, :], res[:, 1, :])

    nc.compile()
    _BUILD_CACHE[key] = nc
    return nc


def kernel(latent_view_1, latent_view_2):
    global LAST_RESULT
    v1 = np.ascontiguousarray(np.asarray(latent_view_1, dtype=np.float32))
    v2 = np.ascontiguousarray(np.asarray(latent_view_2, dtype=np.float32))
    assert v1.shape == (N_FULL, K) and v2.shape == (N_FULL, K)

    nc = _build(MM_MODE)
    in_maps = [
        {
            "v1": v1[c * N_LOC : (c + 1) * N_LOC],
            "v2": v2[c * N_LOC : (c + 1) * N_LOC],
        }
        for c in range(NCORES)
    ]
    LAST_RESULT = bass_utils.run_bass_kernel_spmd(
        nc, in_maps, core_ids=list(range(NCORES))
    )

    # host epilogue in float64 on the tiny [256, 256] joint
    p_i_j = np.zeros((K, K), dtype=np.float64)
    for r in LAST_RESULT.results:
        p_i_j += np.asarray(r["partial"], dtype=np.float64)
    p_i_j = (p_i_j + p_i_j.T) / 2.0
    p_i_j = p_i_j / p_i_j.sum()
    p_i = p_i_j.sum(axis=1, keepdims=True)
    p_j = p_i_j.sum(axis=0, keepdims=True)
    p_i_j = np.maximum(p_i_j, EPS)
    p_i = np.maximum(p_i, EPS)
    p_j = np.maximum(p_j, EPS)
    loss = -(
        p_i_j
        * (
            np.log(p_i_j)
            - (ALPHA + 1.0) * np.log(p_j)
            - (ALPHA + 1.0) * np.log(p_i)
        )
    ).sum()
    return np.array(loss, dtype=np.float32)
